# revision 4
# baseline (speedup 1.0000x reference)
"""Trainium2 Bass kernel for masked dot-product attention.

Problem (hardcoded): B=4, H=16, S=2048, DK=64, fp32 inputs, bool mask [B,1,S,S].
    out = softmax(where(mask, Q K^T, -1e4) / sqrt(DK)) @ V

Sharding: batch*heads = 64 head-slices split across 8 cores (8 heads/core).
Each core owns heads of exactly one batch, so it needs only that batch's mask.

Per-core device kernel (per head, per q-tile of 512):
  - scores computed transposed: T[k,q] = (K^T)^T @ (Q^T), bf16 operands
    (fp32 matmul is 4x slower on the PE), fp32 PSUM.
  - P = exp(T/8) on ScalarE (scale fused into the activation). No max pass:
    logits are ~N(0,1), |logit| < ~10, so exp is safe in fp32.
  - mask applied after exp as a bf16 multiply (VectorE): reference's masked
    logit is -10000/8 -> exp underflows to exactly 0, same as multiplying by 0.
  - PV with stationary V_ext = [V | 1]: out'[d,q] (65 rows) accumulates over
    k-tiles in PSUM; row 64 is the softmax denominator l[q].
  - epilogue: copy out' to SBUF, PE-transpose 128-column chunks, reciprocal of
    l, per-partition scale, DMA out in natural [s,d] layout.

Host side (off-device, numpy): pre-transpose Q,K to [d,s] bf16, mask to
mask^T bf16, V to [k, h, d|1] bf16; gather per-core outputs.
"""

import numpy as np
import ml_dtypes

B, H, S, DK = 4, 16, 2048, 64
NCORES = 8
HPC = H * B // NCORES  # heads per core = 8
QT = 512               # q tile (columns of transposed score tile)
NQT = S // QT          # 4
KT = 128               # k tile (partitions of transposed score tile)
NKT = S // KT          # 16
KPAIR = 2              # k-tiles batched per PSUM tile / exp call
VE = DK + 1            # V extended with a ones column -> denominator row
SCALE = 1.0 / float(np.sqrt(DK))

_BF16 = ml_dtypes.bfloat16

_CACHE = {}


def _build_nc():
    import concourse.mybir as mybir
    import concourse.tile as tile
    from concourse import bacc
    from concourse.masks import make_identity
    from contextlib import ExitStack

    dt = mybir.dt
    nc = bacc.Bacc()

    qT = nc.declare_dram_parameter("qT", [HPC, DK, S], dt.bfloat16, isOutput=False)
    kT = nc.declare_dram_parameter("kT", [HPC, DK, S], dt.bfloat16, isOutput=False)
    vex = nc.declare_dram_parameter("vex", [S, HPC, VE], dt.bfloat16, isOutput=False)
    maskT = nc.declare_dram_parameter("maskT", [S, S], dt.bfloat16, isOutput=False)
    out = nc.declare_dram_parameter("out", [HPC, S, DK], dt.float32, isOutput=True)

    with tile.TileContext(nc) as tc, ExitStack() as ctx:
        const = ctx.enter_context(tc.tile_pool(name="const", bufs=1))
        maskp = ctx.enter_context(tc.tile_pool(name="maskp", bufs=2))
        qp = ctx.enter_context(tc.tile_pool(name="qp", bufs=3))
        pp = ctx.enter_context(tc.tile_pool(name="pp", bufs=2))
        epi = ctx.enter_context(tc.tile_pool(name="epi", bufs=2))
        scps = ctx.enter_context(tc.tile_pool(name="scps", bufs=2, space="PSUM"))
        pvps = ctx.enter_context(tc.tile_pool(name="pvps", bufs=2, space="PSUM"))
        trps = ctx.enter_context(tc.tile_pool(name="trps", bufs=2, space="PSUM"))

        # ---- one-time preloads ----
        ident = const.tile([128, 128], dt.float32)
        make_identity(nc, ident)

        # all heads' K^T cached in SBUF: [64, h, s]
        k_sb = const.tile([DK, HPC, S], dt.bfloat16)
        nc.sync.dma_start(out=k_sb, in_=kT.rearrange("h d s -> d h s"))

        # all heads' V_ext cached in SBUF: [128, kt, h, 65]
        v_sb = const.tile([KT, NKT, HPC, VE], dt.bfloat16)
        nc.sync.dma_start(out=v_sb, in_=vex.rearrange("(j p) h e -> p j h e", p=KT))

        for qt in range(NQT):
            # mask^T column block for this q tile: [128, kt, q] (shared by heads)
            m_sb = maskp.tile([KT, NKT, QT], dt.bfloat16)
            nc.sync.dma_start(
                out=m_sb,
                in_=maskT[:, qt * QT:(qt + 1) * QT].rearrange(
                    "(j p) q -> p j q", p=KT
                ),
            )
            for h in range(HPC):
                # Q^T slice [64, 512]
                q_sb = qp.tile([DK, QT], dt.bfloat16)
                nc.sync.dma_start(out=q_sb, in_=qT[h, :, qt * QT:(qt + 1) * QT])

                p_sb = pp.tile([KT, NKT * QT], dt.bfloat16)
                for i in range(NKT // KPAIR):
                    sc = scps.tile([KT, KPAIR * QT], dt.float32)
                    for u in range(KPAIR):
                        j = i * KPAIR + u
                        nc.tensor.matmul(
                            out=sc[:, u * QT:(u + 1) * QT],
                            lhsT=k_sb[:, h, j * KT:(j + 1) * KT],
                            rhs=q_sb,
                            start=True,
                            stop=True,
                        )
                    # P = exp(scores/8), bf16, into the right column block
                    nc.scalar.activation(
                        out=p_sb[:, i * KPAIR * QT:(i + 1) * KPAIR * QT],
                        in_=sc,
                        func=mybir.ActivationFunctionType.Exp,
                        scale=SCALE,
                    )
                    # apply mask (0/1) -> masked probabilities exactly 0
                    nc.vector.tensor_mul(
                        p_sb[:, i * KPAIR * QT:(i + 1) * KPAIR * QT],
                        p_sb[:, i * KPAIR * QT:(i + 1) * KPAIR * QT],
                        m_sb[:, i * KPAIR:(i + 1) * KPAIR, :].rearrange(
                            "p a q -> p (a q)"
                        ),
                    )

                # PV: out'[65, q] += V_ext[k,:]^T @ P[k, q] over k tiles
                pv = pvps.tile([128, QT], dt.float32)
                for j in range(NKT):
                    nc.tensor.matmul(
                        out=pv[0:VE, :],
                        lhsT=v_sb[:, j, h, :],
                        rhs=p_sb[:, j * QT:(j + 1) * QT],
                        start=(j == 0),
                        stop=(j == NKT - 1),
                    )

                # epilogue: copy to SBUF, transpose 128-col chunks, normalize
                o_sb = epi.tile([VE, QT], dt.float32)
                nc.vector.tensor_copy(o_sb, pv[0:VE, :])

                tr = trps.tile([128, 4 * VE], dt.float32)
                for j in range(4):
                    nc.tensor.transpose(
                        out=tr[:, j * VE:(j + 1) * VE],
                        in_=o_sb[:, j * 128:(j + 1) * 128],
                        identity=ident[0:VE, 0:VE],
                    )
                ot = epi.tile([128, 4, VE], dt.float32)
                nc.vector.tensor_copy(ot, tr.rearrange("p (a e) -> p a e", e=VE))

                rec = epi.tile([128, 4], dt.float32)
                nc.vector.reciprocal(rec, ot[:, :, DK])

                outf = epi.tile([128, 4, DK], dt.float32)
                for j in range(4):
                    nc.vector.tensor_scalar_mul(
                        outf[:, j, :], ot[:, j, 0:DK], rec[:, j:j + 1]
                    )
                nc.sync.dma_start(
                    out=out[h, qt * QT:(qt + 1) * QT, :].rearrange(
                        "(j p) d -> p j d", p=128
                    ),
                    in_=outf,
                )
    nc.compile()
    return nc


def _get_nc():
    if "nc" not in _CACHE:
        _CACHE["nc"] = _build_nc()
    return _CACHE["nc"]


def _prep_core_inputs(q, k, v, m, core):
    b = core // (H // HPC)
    h0 = (core % (H // HPC)) * HPC
    qs = q[b, h0:h0 + HPC]                       # [8, S, DK]
    ks = k[b, h0:h0 + HPC]
    vs = v[b, h0:h0 + HPC]
    qT = qs.transpose(0, 2, 1).astype(_BF16)     # [8, DK, S]
    kT = ks.transpose(0, 2, 1).astype(_BF16)
    vex = np.ones((S, HPC, VE), dtype=_BF16)
    vex[:, :, :DK] = vs.transpose(1, 0, 2)       # [S, 8, DK]
    mT = m[b, 0].T.astype(_BF16)                 # [S(k), S(q)]
    return {"qT": qT, "kT": kT, "vex": vex, "maskT": np.ascontiguousarray(mT)}


def kernel(query, key, value, mask):
    from concourse.bass_utils import run_bass_kernel_spmd

    q = np.asarray(query, dtype=np.float32)
    k = np.asarray(key, dtype=np.float32)
    v = np.asarray(value, dtype=np.float32)
    m = np.asarray(mask)

    nc = _get_nc()
    in_maps = [_prep_core_inputs(q, k, v, m, c) for c in range(NCORES)]
    res = run_bass_kernel_spmd(nc, in_maps, list(range(NCORES))).results

    out = np.empty((B, H, S, DK), dtype=np.float32)
    for c in range(NCORES):
        b = c // (H // HPC)
        h0 = (c % (H // HPC)) * HPC
        out[b, h0:h0 + HPC] = res[c]["out"]
    return out


# revision 7
# speedup vs baseline: 1.1138x; 1.1138x over previous
"""Trainium2 Bass kernel for masked dot-product attention.

Problem (hardcoded): B=4, H=16, S=2048, DK=64, fp32 inputs, bool mask [B,1,S,S].
    out = softmax(where(mask, Q K^T, -1e4) / sqrt(DK)) @ V

Sharding: batch*heads = 64 head-slices split across 8 cores (8 heads/core).
Each core owns heads of exactly one batch, so it needs only that batch's mask.

Per-core device kernel (per head, per q-tile of 512):
  - scores computed transposed: T[k,q] = (K^T)^T @ (Q^T), bf16 operands
    (fp32 matmul is 4x slower on the PE), fp32 PSUM.
  - P = exp(T/8) on ScalarE (scale fused into the activation). No max pass:
    logits are ~N(0,1), |logit| < ~10, so exp is safe in fp32.
  - mask applied after exp as a bf16 multiply (VectorE): reference's masked
    logit is -10000/8 -> exp underflows to exactly 0, same as multiplying by 0.
  - PV with stationary V_ext = [V | 1]: out'[d,q] (65 rows) accumulates over
    k-tiles in PSUM; row 64 is the softmax denominator l[q].
  - epilogue: copy out' to SBUF, PE-transpose 128-column chunks, reciprocal of
    l, per-partition scale, DMA out in natural [s,d] layout.

Host side (off-device, numpy): pre-transpose Q,K to [d,s] bf16, mask to
mask^T bf16, V to [k, h, d|1] bf16; gather per-core outputs.
"""

import numpy as np
import ml_dtypes

B, H, S, DK = 4, 16, 2048, 64
NCORES = 8
HPC = H * B // NCORES  # heads per core = 8
QT = 512               # q tile (columns of transposed score tile)
NQT = S // QT          # 4
KT = 128               # k tile (partitions of transposed score tile)
NKT = S // KT          # 16
KPAIR = 2              # k-tiles batched per PSUM tile / exp call
VE = DK + 1            # V extended with a ones column -> denominator row
SCALE = 1.0 / float(np.sqrt(DK))

_BF16 = ml_dtypes.bfloat16

_CACHE = {}


def _build_nc(reps=1):
    import concourse.mybir as mybir
    import concourse.tile as tile
    from concourse import bacc
    from concourse.masks import make_identity
    from contextlib import ExitStack

    dt = mybir.dt
    nc = bacc.Bacc()

    qT = nc.declare_dram_parameter("qT", [HPC, DK, S], dt.bfloat16, isOutput=False)
    kT = nc.declare_dram_parameter("kT", [HPC, DK, S], dt.bfloat16, isOutput=False)
    vex = nc.declare_dram_parameter("vex", [S, HPC, VE], dt.bfloat16, isOutput=False)
    maskT = nc.declare_dram_parameter("maskT", [S, S], dt.bfloat16, isOutput=False)
    out = nc.declare_dram_parameter("out", [HPC, S, DK], dt.float32, isOutput=True)

    with tile.TileContext(nc) as tc, ExitStack() as ctx:
        const = ctx.enter_context(tc.tile_pool(name="const", bufs=1))
        maskp = ctx.enter_context(tc.tile_pool(name="maskp", bufs=2))
        qp = ctx.enter_context(tc.tile_pool(name="qp", bufs=3))
        pp = ctx.enter_context(tc.tile_pool(name="pp", bufs=2))
        epi = ctx.enter_context(tc.tile_pool(name="epi", bufs=2))
        scps = ctx.enter_context(tc.tile_pool(name="scps", bufs=2, space="PSUM"))
        pvps = ctx.enter_context(tc.tile_pool(name="pvps", bufs=2, space="PSUM"))
        trps = ctx.enter_context(tc.tile_pool(name="trps", bufs=2, space="PSUM"))

        # ---- one-time preloads ----
        ident = const.tile([128, 128], dt.float32)
        make_identity(nc, ident)

        # all heads' K^T cached in SBUF: [64, h, s]
        k_sb = const.tile([DK, HPC, S], dt.bfloat16)
        nc.sync.dma_start(out=k_sb, in_=kT.rearrange("h d s -> d h s"))

        # all heads' V_ext cached in SBUF: [128, kt, h, 65]
        v_sb = const.tile([KT, NKT, HPC, VE], dt.bfloat16)
        nc.sync.dma_start(out=v_sb, in_=vex.rearrange("(j p) h e -> p j h e", p=KT))

        for _rep in range(reps):
         for qt in range(NQT):
            # mask^T column block for this q tile: [128, kt, q] (shared by heads)
            m_sb = maskp.tile([KT, NKT, QT], dt.bfloat16)
            nc.sync.dma_start(
                out=m_sb,
                in_=maskT[:, qt * QT:(qt + 1) * QT].rearrange(
                    "(j p) q -> p j q", p=KT
                ),
            )
            for h in range(HPC):
                # Q^T slice [64, 512]
                q_sb = qp.tile([DK, QT], dt.bfloat16)
                nc.sync.dma_start(out=q_sb, in_=qT[h, :, qt * QT:(qt + 1) * QT])

                p_sb = pp.tile([KT, NKT * QT], dt.bfloat16)
                for i in range(NKT // KPAIR):
                    sc = scps.tile([KT, KPAIR * QT], dt.float32)
                    for u in range(KPAIR):
                        j = i * KPAIR + u
                        nc.tensor.matmul(
                            out=sc[:, u * QT:(u + 1) * QT],
                            lhsT=k_sb[:, h, j * KT:(j + 1) * KT],
                            rhs=q_sb,
                            start=True,
                            stop=True,
                        )
                    # P = exp(scores/8), bf16, into the right column block
                    nc.scalar.activation(
                        out=p_sb[:, i * KPAIR * QT:(i + 1) * KPAIR * QT],
                        in_=sc,
                        func=mybir.ActivationFunctionType.Exp,
                        scale=SCALE,
                    )
                    # apply mask (0/1) -> masked probabilities exactly 0
                    nc.vector.tensor_mul(
                        p_sb[:, i * KPAIR * QT:(i + 1) * KPAIR * QT],
                        p_sb[:, i * KPAIR * QT:(i + 1) * KPAIR * QT],
                        m_sb[:, i * KPAIR:(i + 1) * KPAIR, :].rearrange(
                            "p a q -> p (a q)"
                        ),
                    )

                # PV: out'[65, q] += V_ext[k,:]^T @ P[k, q] over k tiles
                pv = pvps.tile([128, QT], dt.float32)
                for j in range(NKT):
                    nc.tensor.matmul(
                        out=pv[0:VE, :],
                        lhsT=v_sb[:, j, h, :],
                        rhs=p_sb[:, j * QT:(j + 1) * QT],
                        start=(j == 0),
                        stop=(j == NKT - 1),
                    )

                # epilogue: copy to SBUF, transpose 128-col chunks, normalize
                o_sb = epi.tile([VE, QT], dt.float32)
                nc.vector.tensor_copy(o_sb, pv[0:VE, :])

                tr = trps.tile([128, 4 * VE], dt.float32)
                for j in range(4):
                    nc.tensor.transpose(
                        out=tr[:, j * VE:(j + 1) * VE],
                        in_=o_sb[:, j * 128:(j + 1) * 128],
                        identity=ident[0:VE, 0:VE],
                    )
                ot = epi.tile([128, 4, VE], dt.float32)
                nc.vector.tensor_copy(ot, tr.rearrange("p (a e) -> p a e", e=VE))

                rec = epi.tile([128, 4], dt.float32)
                nc.vector.reciprocal(rec, ot[:, :, DK])

                outf = epi.tile([128, 4, DK], dt.float32)
                for j in range(4):
                    nc.vector.tensor_scalar_mul(
                        outf[:, j, :], ot[:, j, 0:DK], rec[:, j:j + 1]
                    )
                nc.sync.dma_start(
                    out=out[h, qt * QT:(qt + 1) * QT, :].rearrange(
                        "(j p) d -> p j d", p=128
                    ),
                    in_=outf,
                )
    nc.compile()
    return nc


def _get_nc(reps=1):
    key = ("nc", reps)
    if key not in _CACHE:
        _CACHE[key] = _build_nc(reps)
    return _CACHE[key]


def _prep_core_inputs(q, k, v, m, core):
    b = core // (H // HPC)
    h0 = (core % (H // HPC)) * HPC
    qs = q[b, h0:h0 + HPC]                       # [8, S, DK]
    ks = k[b, h0:h0 + HPC]
    vs = v[b, h0:h0 + HPC]
    qT = qs.transpose(0, 2, 1).astype(_BF16)     # [8, DK, S]
    kT = ks.transpose(0, 2, 1).astype(_BF16)
    vex = np.ones((S, HPC, VE), dtype=_BF16)
    vex[:, :, :DK] = vs.transpose(1, 0, 2)       # [S, 8, DK]
    mT = m[b, 0].T.astype(_BF16)                 # [S(k), S(q)]
    return {"qT": qT, "kT": kT, "vex": vex, "maskT": np.ascontiguousarray(mT)}


def kernel(query, key, value, mask):
    from concourse.bass_utils import run_bass_kernel_spmd

    q = np.asarray(query, dtype=np.float32)
    k = np.asarray(key, dtype=np.float32)
    v = np.asarray(value, dtype=np.float32)
    m = np.asarray(mask)

    nc = _get_nc()
    in_maps = [_prep_core_inputs(q, k, v, m, c) for c in range(NCORES)]
    res = run_bass_kernel_spmd(nc, in_maps, list(range(NCORES))).results

    out = np.empty((B, H, S, DK), dtype=np.float32)
    for c in range(NCORES):
        b = c // (H // HPC)
        h0 = (c % (H // HPC)) * HPC
        out[b, h0:h0 + HPC] = res[c]["out"]
    return out


# revision 10
# speedup vs baseline: 1.4133x; 1.2689x over previous
"""Trainium2 Bass kernel for masked dot-product attention.

Problem (hardcoded): B=4, H=16, S=2048, DK=64, fp32 inputs, bool mask [B,1,S,S].
    out = softmax(where(mask, Q K^T, -1e4) / sqrt(DK)) @ V

Sharding: batch*heads = 64 head-slices split across 8 cores (8 heads/core).
Each core owns heads of exactly one batch, so it needs only that batch's mask.

Per-core device kernel (per head, per q-tile of 512):
  - scores computed transposed: T[k,q] = (K^T)^T @ (Q^T), bf16 operands
    (fp32 matmul is 4x slower on the PE), fp32 PSUM.
  - P = exp(T/8) on ScalarE (scale fused into the activation). No max pass:
    logits are ~N(0,1), |logit| < ~10, so exp is safe in fp32.
  - mask applied after exp as a bf16 multiply (VectorE): reference's masked
    logit is -10000/8 -> exp underflows to exactly 0, same as multiplying by 0.
  - PV with stationary V_ext = [V | 1]: out'[d,q] (65 rows) accumulates over
    k-tiles in PSUM; row 64 is the softmax denominator l[q].
  - epilogue: copy out' to SBUF, PE-transpose 128-column chunks, reciprocal of
    l, per-partition scale, DMA out in natural [s,d] layout.

Host side (off-device, numpy): pre-transpose Q,K to [d,s] bf16, mask to
mask^T bf16, V to [k, h, d|1] bf16; gather per-core outputs.
"""

import numpy as np
import ml_dtypes

B, H, S, DK = 4, 16, 2048, 64
NCORES = 8
HPC = H * B // NCORES  # heads per core = 8
QT = 512               # q tile (columns of transposed score tile)
NQT = S // QT          # 4
KT = 128               # k tile (partitions of transposed score tile)
NKT = S // KT          # 16
KPAIR = 2              # k-tiles batched per PSUM tile / exp call
VE = DK + 1            # V extended with a ones column -> denominator row
SCALE = 1.0 / float(np.sqrt(DK))

_BF16 = ml_dtypes.bfloat16

_CACHE = {}


def _build_nc(reps=1):
    import concourse.mybir as mybir
    import concourse.tile as tile
    from concourse import bacc
    from concourse.masks import make_identity
    from contextlib import ExitStack

    dt = mybir.dt
    nc = bacc.Bacc()

    qT = nc.declare_dram_parameter("qT", [HPC, DK, S], dt.bfloat16, isOutput=False)
    kT = nc.declare_dram_parameter("kT", [HPC, DK, S], dt.bfloat16, isOutput=False)
    vex = nc.declare_dram_parameter("vex", [S, HPC, VE], dt.bfloat16, isOutput=False)
    maskT = nc.declare_dram_parameter("maskT", [S, S], dt.bfloat16, isOutput=False)
    out = nc.declare_dram_parameter("out", [HPC, S, DK], dt.float32, isOutput=True)

    with tile.TileContext(nc) as tc, ExitStack() as ctx:
        const = ctx.enter_context(tc.tile_pool(name="const", bufs=1))
        maskp = ctx.enter_context(tc.tile_pool(name="maskp", bufs=2))
        qp = ctx.enter_context(tc.tile_pool(name="qp", bufs=3))
        pp = ctx.enter_context(tc.tile_pool(name="pp", bufs=3))
        epi = ctx.enter_context(tc.tile_pool(name="epi", bufs=3))
        scps = ctx.enter_context(tc.tile_pool(name="scps", bufs=2, space="PSUM"))
        pvps = ctx.enter_context(tc.tile_pool(name="pvps", bufs=2, space="PSUM"))
        trps = ctx.enter_context(tc.tile_pool(name="trps", bufs=2, space="PSUM"))

        # ---- one-time preloads ----
        ident = const.tile([128, 128], dt.float32)
        make_identity(nc, ident)

        # trigger the ACT exp table load early so it overlaps the preload DMAs
        warm = const.tile([1, 2], dt.float32)
        nc.vector.memset(warm, 0.0)
        nc.scalar.activation(out=warm, in_=warm,
                             func=mybir.ActivationFunctionType.Exp)

        # all heads' K^T cached in SBUF: [64, h, s]; head 0 first so the
        # first scores matmul can start before the bulk preload finishes
        k_sb = const.tile([DK, HPC, S], dt.bfloat16)
        nc.sync.dma_start(out=k_sb[:, 0, :], in_=kT[0])
        nc.sync.dma_start(
            out=k_sb[:, 1:, :], in_=kT[1:].rearrange("h d s -> d h s")
        )

        # all heads' V_ext cached in SBUF: [128, kt, h, 65]
        v_sb = const.tile([KT, NKT, HPC, VE], dt.bfloat16)
        nc.sync.dma_start(out=v_sb, in_=vex.rearrange("(j p) h e -> p j h e", p=KT))

        for _rep in range(reps):
         for qt in range(NQT):
            # mask^T column block for this q tile: [128, kt, q] (shared by heads)
            m_sb = maskp.tile([KT, NKT, QT], dt.bfloat16)
            nc.sync.dma_start(
                out=m_sb,
                in_=maskT[:, qt * QT:(qt + 1) * QT].rearrange(
                    "(j p) q -> p j q", p=KT
                ),
            )
            for h in range(HPC):
                # Q^T slice [64, 512]
                q_sb = qp.tile([DK, QT], dt.bfloat16)
                nc.sync.dma_start(out=q_sb, in_=qT[h, :, qt * QT:(qt + 1) * QT])

                p_sb = pp.tile([KT, NKT * QT], dt.bfloat16)
                j0 = 0
                for grp in (2, 2, 2, 2, 2, 2, 2, 2):
                    sc = scps.tile([KT, 2 * QT], dt.float32, tag="sc")
                    for u in range(grp):
                        j = j0 + u
                        nc.tensor.matmul(
                            out=sc[:, u * QT:(u + 1) * QT],
                            lhsT=k_sb[:, h, j * KT:(j + 1) * KT],
                            rhs=q_sb,
                            start=True,
                            stop=True,
                        )
                    # P = exp(scores/8), bf16, into the right column block
                    nc.scalar.activation(
                        out=p_sb[:, j0 * QT:(j0 + grp) * QT],
                        in_=sc[:, 0:grp * QT],
                        func=mybir.ActivationFunctionType.Exp,
                        scale=SCALE,
                    )
                    # apply mask (0/1) -> masked probabilities exactly 0
                    nc.vector.tensor_mul(
                        p_sb[:, j0 * QT:(j0 + grp) * QT],
                        p_sb[:, j0 * QT:(j0 + grp) * QT],
                        m_sb[:, j0:j0 + grp, :].rearrange("p a q -> p (a q)"),
                    )
                    j0 += grp

                # PV: out'[65, q] += V_ext[k,:]^T @ P[k, q] over k tiles
                pv = pvps.tile([128, QT], dt.float32)
                for j in range(NKT):
                    nc.tensor.matmul(
                        out=pv[0:VE, :],
                        lhsT=v_sb[:, j, h, :],
                        rhs=p_sb[:, j * QT:(j + 1) * QT],
                        start=(j == 0),
                        stop=(j == NKT - 1),
                    )

                # epilogue: copy to SBUF, transpose 128-col chunks, normalize
                o_sb = epi.tile([VE, QT], dt.float32)
                nc.vector.tensor_copy(o_sb, pv[0:VE, :])

                tr = trps.tile([128, 4 * VE], dt.float32)
                for j in range(4):
                    nc.tensor.transpose(
                        out=tr[:, j * VE:(j + 1) * VE],
                        in_=o_sb[:, j * 128:(j + 1) * 128],
                        identity=ident[0:VE, 0:VE],
                    )
                ot = epi.tile([128, 4, VE], dt.float32)
                nc.vector.tensor_copy(ot, tr.rearrange("p (a e) -> p a e", e=VE))

                rec = epi.tile([128, 4], dt.float32)
                nc.vector.reciprocal(rec, ot[:, :, DK])

                outf = epi.tile([128, 4, DK], dt.float32)
                for j in range(4):
                    nc.vector.tensor_scalar_mul(
                        outf[:, j, :], ot[:, j, 0:DK], rec[:, j:j + 1]
                    )
                nc.sync.dma_start(
                    out=out[h, qt * QT:(qt + 1) * QT, :].rearrange(
                        "(j p) d -> p j d", p=128
                    ),
                    in_=outf,
                )
    nc.compile()
    return nc


def _get_nc(reps=1):
    key = ("nc", reps)
    if key not in _CACHE:
        _CACHE[key] = _build_nc(reps)
    return _CACHE[key]


def _prep_core_inputs(q, k, v, m, core):
    b = core // (H // HPC)
    h0 = (core % (H // HPC)) * HPC
    qs = q[b, h0:h0 + HPC]                       # [8, S, DK]
    ks = k[b, h0:h0 + HPC]
    vs = v[b, h0:h0 + HPC]
    qT = qs.transpose(0, 2, 1).astype(_BF16)     # [8, DK, S]
    kT = ks.transpose(0, 2, 1).astype(_BF16)
    vex = np.ones((S, HPC, VE), dtype=_BF16)
    vex[:, :, :DK] = vs.transpose(1, 0, 2)       # [S, 8, DK]
    mT = m[b, 0].T.astype(_BF16)                 # [S(k), S(q)]
    return {"qT": qT, "kT": kT, "vex": vex, "maskT": np.ascontiguousarray(mT)}


def kernel(query, key, value, mask):
    from concourse.bass_utils import run_bass_kernel_spmd

    q = np.asarray(query, dtype=np.float32)
    k = np.asarray(key, dtype=np.float32)
    v = np.asarray(value, dtype=np.float32)
    m = np.asarray(mask)

    nc = _get_nc()
    in_maps = [_prep_core_inputs(q, k, v, m, c) for c in range(NCORES)]
    res = run_bass_kernel_spmd(nc, in_maps, list(range(NCORES))).results

    out = np.empty((B, H, S, DK), dtype=np.float32)
    for c in range(NCORES):
        b = c // (H // HPC)
        h0 = (c % (H // HPC)) * HPC
        out[b, h0:h0 + HPC] = res[c]["out"]
    return out
